# revision 14
# baseline (speedup 1.0000x reference)
"""ARMA GNN (single-layer ARMAConv + residual) as a distributed Bass kernel
on 8 TRN2 NeuronCores.

Math (reference):
    deg[d]   = #incoming edges of d;  dinv = deg^-1/2 (0 where deg==0)
    w[e]     = dinv[src_e] * dinv[dst_e]
    xa       = A_hat @ x                (segment-sum of w[e] * x[src_e] into dst_e)
    y_k      = xa @ W_k + x @ V_k + b_k          (assoc: A@(xW) == (A@x)@W)
    out      = x + relu(mean_k relu(y_k)) = x + 0.5*relu(y_0) + 0.5*relu(y_1)
               (outer relu is a no-op on a nonneg sum of relus)

Distribution: nodes are degree-balanced across 8 cores (and across 128-row
tiles within a core). Each core owns its destination nodes and every edge
pointing into them; the halo (x rows its edges read) is shipped per edge
slot, pre-scaled by w_e, as a contiguous wrapped table streamed by plain
DMA. One-hot(dst-position) selection matrices turn the segment-sum into
TensorE matmuls accumulating xa^T tiles in PSUM. The dense matmuls run
entirely in fp8 DoubleRow (2 contraction rows/cycle): z^T = Wq^T [x8;xa8]^T
with x8 = fp8(x/4), xa8 = fp8(xa/4) (scaled PSUM->fp8 copy), Wq = fp8(2W),
so PSUM holds 0.5*y_k exactly and the epilogue is two fused relu+add
(scalar_tensor_tensor max/add) per output half, no extra scaling pass.
One-hot tables for the first chunks are precomputed host-side and DMA'd so
the first matmul does not wait on the gpsimd scatter library warmup.
"""

import sys

for _p in ("/opt/trn_rl_repo", "/opt/pypackages"):
    if _p not in sys.path:
        sys.path.append(_p)

import numpy as np
import ml_dtypes

import concourse.tile as tile
from concourse import bacc, library_config, mybir
from concourse.bass_utils import run_bass_kernel_spmd

BF16 = ml_dtypes.bfloat16
F8 = ml_dtypes.float8_e4m3

# Problem constants (nn_Arma_83330955477199)
N, E, F, K = 50000, 320000, 256, 2
N_CORES = 8
P = 128

# Per-core geometry
NL = N // N_CORES                                # 6250 real nodes per core
N_TILES = (NL + P - 1) // P                      # 49 tiles
NLP = N_TILES * P                                # 6272 padded rows

MAX_CHUNK_GROUPS = 15   # local_scatter cap: num_elems = 15*128 < 2047
NPRE = 3                # leading single-tile chunks with host-built eq
LOOKAHEAD = 5           # gather-DMA prefetch depth (chunks)

XS = 0.25               # fp8 scale on x / xa
WS = 2.0                # fp8 scale on weights  (XS*WS = 0.5 = mean over K)


# --------------------------------------------------------------------------
# Host-side preprocessing: graph partitioning + layout prep
# --------------------------------------------------------------------------

def _preprocess(x, edge_index, init_weight, root_weight, bias):
    src = np.asarray(edge_index[0], dtype=np.int64)
    dst = np.asarray(edge_index[1], dtype=np.int64)
    x = np.asarray(x, dtype=np.float32)

    deg = np.bincount(dst, minlength=N).astype(np.float32)
    dinv = np.where(deg > 0, 1.0 / np.sqrt(np.maximum(deg, 1.0)), 0.0).astype(
        np.float32
    )

    # --- node -> (core, tile, pos): snake-deal by degree for edge balance
    order = np.argsort(-deg, kind="stable")
    core_of = np.empty(N, dtype=np.int32)
    loc_of = np.empty(N, dtype=np.int32)
    n_rounds = N // N_CORES
    fwd = np.arange(N_CORES)
    snake = np.empty((n_rounds, N_CORES), dtype=np.int64)
    snake[0::2] = fwd
    snake[1::2] = fwd[::-1]
    core_of[order] = snake.reshape(-1)
    # within each core: degree-aware bin packing of nodes into tiles so the
    # per-tile edge counts land just under aligned group budgets (the same
    # hi/lo tile split on every core, since the schedule takes the max)
    degi = deg.astype(np.int64)
    percore_e = np.array(
        [degi[core_of == r].sum() for r in range(N_CORES)]
    )
    C_hi, C_lo = 7 * P, 6 * P
    n_hi = int(np.ceil((percore_e.max() + 192 - N_TILES * C_lo) / P))
    n_hi = min(max(n_hi, 0), N_TILES)
    ecaps = np.full(N_TILES, C_lo, dtype=np.int64)
    ecaps[:n_hi] = C_hi
    for r in range(N_CORES):
        nodes_r = order[core_of[order] == r]  # degree-sorted desc
        assert len(nodes_r) == NL
        degs = degi[nodes_r]
        bsum = np.zeros(N_TILES, dtype=np.int64)
        bn = np.zeros(N_TILES, dtype=np.int64)
        for i in range(NL):
            dg = degs[i]
            room = ecaps - bsum
            slots = P - bn
            fit = (slots > 0) & (room >= dg)
            if fit.any():
                sc = np.where(fit, room / np.maximum(slots, 1), -1e18)
                b = int(np.argmax(sc))
            else:
                sc = np.where(slots > 0, (room - dg).astype(float), -1e18)
                b = int(np.argmax(sc))
            loc_of[nodes_r[i]] = b * P + bn[b]
            bsum[b] += dg
            bn[b] += 1

    # --- per-core edge lists (owned by dst core)
    e_core = core_of[dst]
    per_core = []
    for r in range(N_CORES):
        m = e_core == r
        s_r, d_r = src[m], dst[m]
        d_loc = loc_of[d_r]
        per_core.append((s_r, d_r, d_loc // P, d_loc % P))

    # --- static schedule: groups per tile = max over cores
    cnt = np.zeros((N_CORES, N_TILES), dtype=np.int64)
    for r in range(N_CORES):
        cnt[r] = np.bincount(per_core[r][2], minlength=N_TILES)
    g_per_tile = np.maximum(1, (cnt.max(axis=0) + P - 1) // P).astype(np.int64)
    G = int(g_per_tile.sum())
    EG = G * P
    slot_base = np.concatenate([[0], np.cumsum(g_per_tile * P)])

    # --- gather/compute chunks: pack tiles with <= MAX_CHUNK_GROUPS groups
    chunks = [(0, 1), (1, 2), (2, 3)]
    lo = 3
    while lo < N_TILES:
        hi = lo
        gacc = 0
        while hi < N_TILES and (
            gacc + g_per_tile[hi] <= MAX_CHUNK_GROUPS or hi == lo
        ):
            gacc += int(g_per_tile[hi])
            hi += 1
        chunks.append((lo, hi))
        lo = hi
    max_chunk_groups = max(
        int(slot_base[hi] - slot_base[lo]) // P for lo, hi in chunks
    )
    # chunk-local scatter index base per group + even-aligned column bases
    g_chunk_base = np.zeros(G, dtype=np.int64)
    col_base = []
    cols = 0
    for lo, hi in chunks:
        g0 = int(slot_base[lo]) // P
        g1 = int(slot_base[hi]) // P
        g_chunk_base[g0:g1] = g0
        col_base.append(cols)
        ng = g1 - g0
        cols += ng + (ng % 2)  # always even per chunk -> 4B-aligned bases
    scat_cols = cols

    npre_g = int(slot_base[NPRE]) // P  # groups covered by host-built eq

    # --- per-core device inputs
    in_maps = []
    for r in range(N_CORES):
        s_r, d_r, t_r, p_r = per_core[r]

        slot_src = np.zeros(EG, dtype=np.int64)
        slot_pos = np.full(EG, -1, dtype=np.int64)
        slot_w = np.zeros(EG, dtype=np.float32)
        eorder = np.argsort(t_r, kind="stable")
        ts_sorted = t_r[eorder]
        starts = np.searchsorted(ts_sorted, np.arange(N_TILES))
        ends = np.searchsorted(ts_sorted, np.arange(N_TILES) + 1)
        for t in range(N_TILES):
            es = eorder[starts[t]:ends[t]]
            b = slot_base[t]
            slot_src[b:b + len(es)] = s_r[es]
            slot_pos[b:b + len(es)] = p_r[es]
            slot_w[b:b + len(es)] = dinv[d_r[es]] * dinv[s_r[es]]

        # local_scatter: one-hot stored as fp8 byte pairs in 16-bit cells;
        # cell = (group - chunk_base)*64 + dstpos//2, byte = dstpos parity.
        # cols [0, scat_cols) are indices, [scat_cols, 2*scat_cols) data.
        gidx = np.arange(G)
        pos = slot_pos.reshape(G, P)
        sidx = np.where(
            pos >= 0,
            (gidx - g_chunk_base)[:, None] * 64 + pos // 2,
            -1,
        ).astype(np.int16)  # [G, 128]
        sdat = np.where(pos % 2 == 0, 0x0038, 0x3800).astype(np.int16)
        scatidx = np.full((P, 2 * scat_cols), -1, dtype=np.int16)
        scatidx[:, scat_cols:] = 0  # data half: pads must stay finite as bf16
        for ci, (clo, chi) in enumerate(chunks):
            cg0 = int(slot_base[clo]) // P
            cg1 = int(slot_base[chi]) // P
            cb = col_base[ci]
            scatidx[:, cb : cb + cg1 - cg0] = sidx[cg0:cg1].T
            scatidx[:, scat_cols + cb : scat_cols + cb + cg1 - cg0] = (
                sdat[cg0:cg1].T
            )

        # host-materialized eq for the first NPRE chunks (what local_scatter
        # would produce: zeros + one fp8(1.0) byte per real slot)
        eq01 = np.zeros((P, npre_g, 64), dtype=np.int16)
        for g in range(npre_g):
            pg = pos[g]  # [128] dst positions (or -1)
            for p in range(P):
                if pg[p] >= 0:
                    eq01[p, g, pg[p] // 2] = (
                        0x0038 if pg[p] % 2 == 0 else 0x3800
                    )

        # per-slot message rows, wrapped so each partition's stream is
        # contiguous in DRAM: slots[p, c, :] = row (c*128 + p)
        slots = np.ascontiguousarray(
            (x[slot_src] * slot_w[:, None])
            .astype(F8)
            .reshape(G, P, F)
            .transpose(1, 0, 2)
        )  # [128, G, F]

        mine = np.where(core_of == r)[0]
        x_core = np.zeros((NLP, F), dtype=np.float32)
        x_core[loc_of[mine]] = x[mine]
        xcT = np.ascontiguousarray(
            x_core.T.reshape(2, P, NLP).transpose(1, 0, 2)
        )  # [p, block, m] fp32; f = block*128 + p
        xres = xcT.astype(BF16)

        in_maps.append(
            {
                "slots": slots,
                "scatidx": scatidx,
                "xres": xres,
                "eq01": eq01.view(BF16),
            }
        )

    # replicated fp8 weights: wq[p, z*4 + nt*2 + part, jj, n]
    #   = WS * Wpart_z[jj*128 + p, nt*128 + n]   (part 0 = root/x, 1 = init/xa)
    wq = np.zeros((P, 8, 2, P), dtype=np.float32)
    for z in range(K):
        for part, wmat in ((0, root_weight[z]), (1, init_weight[z])):
            wm = np.asarray(wmat, dtype=np.float32)
            for nt in range(2):
                for jj in range(2):
                    wq[:, z * 4 + nt * 2 + part, jj, :] = (
                        WS * wm[jj * P : (jj + 1) * P, nt * P : (nt + 1) * P]
                    )
    wq = np.ascontiguousarray(wq).astype(F8)

    bias_np = np.asarray(bias, dtype=np.float32)
    has_bias = bool(np.any(bias_np != 0.0))
    assert not has_bias, "nonzero bias not implemented (reference uses zeros)"

    for m in in_maps:
        m["wq"] = wq

    meta = {
        "g_per_tile": g_per_tile,
        "slot_base": slot_base,
        "G": G,
        "EG": EG,
        "chunks": chunks,
        "col_base": col_base,
        "scat_cols": scat_cols,
        "max_chunk_groups": max_chunk_groups,
        "npre_g": npre_g,
        "core_of": core_of,
        "loc_of": loc_of,
    }
    return in_maps, meta


# --------------------------------------------------------------------------
# Device kernel builder
# --------------------------------------------------------------------------

def _build(meta):
    g_per_tile = meta["g_per_tile"]
    slot_base = meta["slot_base"]
    G, EG = meta["G"], meta["EG"]
    chunks = meta["chunks"]
    col_base = meta["col_base"]
    scat_cols = meta["scat_cols"]
    mcg = meta["max_chunk_groups"]
    npre_g = meta["npre_g"]

    nc = bacc.Bacc(
        "TRN2", target_bir_lowering=False, debug=False, num_devices=N_CORES
    )
    bf16 = mybir.dt.bfloat16
    f32 = mybir.dt.float32
    f8 = mybir.dt.float8e4
    i16 = mybir.dt.int16
    DR = mybir.MatmulPerfMode.DoubleRow
    MAX = mybir.AluOpType.max
    ADD = mybir.AluOpType.add

    slots = nc.declare_dram_parameter("slots", [P, G, F], f8, isOutput=False)
    scatidx = nc.declare_dram_parameter(
        "scatidx", [P, 2 * scat_cols], i16, isOutput=False
    )
    xres = nc.declare_dram_parameter("xres", [P, 2, NLP], bf16, isOutput=False)
    wq = nc.declare_dram_parameter("wq", [P, 8, 2, P], f8, isOutput=False)
    eq01 = nc.declare_dram_parameter(
        "eq01", [P, npre_g, 64], bf16, isOutput=False
    )
    out = nc.declare_dram_parameter("out", [P, 2, NLP], bf16, isOutput=True)

    n_chunks = len(chunks)

    def chunk_g(ci):
        lo, hi = chunks[ci]
        s0, s1 = int(slot_base[lo]), int(slot_base[hi])
        return s0 // P, (s1 - s0) // P  # (first group, #groups)

    with tile.TileContext(nc) as tc:
        with (
            tc.tile_pool(name="const", bufs=1) as cpool,
            tc.tile_pool(name="gath", bufs=LOOKAHEAD + 1) as gpool,
            tc.tile_pool(name="eq", bufs=4) as epool,
            tc.tile_pool(name="work", bufs=3) as wpool,
            tc.tile_pool(name="psA", bufs=2, space="PSUM") as psa_pool,
            tc.tile_pool(name="psZ", bufs=1, space="PSUM") as psz_pool,
        ):
            nc.gpsimd.load_library(library_config.local_scatter)

            scatidx_sb = cpool.tile([P, 2 * scat_cols], i16)
            wq_sb = cpool.tile([P, 8, 2, P], f8)
            xres_sb = cpool.tile([P, 2, NLP], bf16)
            xT8_sb = cpool.tile([P, 2, NLP], f8)
            xaT8_sb = cpool.tile([P, 2, NLP], f8)

            gath_tiles = {}

            def issue_gath(ci):
                if ci >= n_chunks:
                    return
                g0, ng = chunk_g(ci)
                gt = gpool.tile([P, mcg, F], f8, tag="gath")
                nc.sync.dma_start(gt[:, :ng, :], slots[:, g0 : g0 + ng, :])
                gath_tiles[ci] = gt

            # xres streamed in m-chunk-aligned pieces; x8 = fp8(XS * x)
            # derived on chip one m-chunk ahead of its dense matmuls
            XQ = [(0, 1536), (1536, 3072), (3072, 4608), (4608, NLP)]
            xq_of_mc = lambda mc: min(mc // 3, 3)

            def issue_xres(q):
                qs, qe = XQ[q]
                nc.scalar.dma_start(
                    xres_sb[:, :, qs:qe], xres[:, :, qs:qe]
                )

            def make_x8(mc):
                ms = mc * 512
                mw = min(512, NLP - ms)
                nc.scalar.activation(
                    xT8_sb[:, :, ms : ms + mw],
                    xres_sb[:, :, ms : ms + mw],
                    mybir.ActivationFunctionType.Copy,
                    scale=XS,
                )

            # priority startup: gathers on sync (alone on that queue),
            # everything else on scalar
            for ci in range(LOOKAHEAD):
                issue_gath(ci)
            eq_tiles = {}
            for ci in range(min(NPRE, n_chunks)):
                g0, ng = chunk_g(ci)
                et = epool.tile([P, mcg, 64], bf16, tag="eq")
                nc.scalar.dma_start(
                    et[:, :ng, :], eq01[:, g0 : g0 + ng, :]
                )
                eq_tiles[ci] = et
            nc.scalar.dma_start(scatidx_sb[:], scatidx[:, :])
            nc.scalar.dma_start(wq_sb[:], wq[:, :, :, :])
            issue_xres(0)

            # junk matmuls bridge the DMA preamble so the PE activity
            # monitor unthrottles the clock just as real work arrives
            warm = cpool.tile([P, 64], f8)
            nc.vector.memset(warm[:], 0)
            wps = psz_pool.tile([P, 512], f32, space="PSUM", tag="psz_0_0")
            for _ in range(48):
                nc.tensor.matmul(
                    out=wps[0:64, 0:64],
                    lhsT=warm[:, 0:64],
                    rhs=warm[:, :],
                    start=True,
                    stop=True,
                )


            for ci in range(n_chunks):
                lo, hi = chunks[ci]
                g0, ng = chunk_g(ci)

                issue_gath(ci + LOOKAHEAD)
                if ci == 2:
                    issue_xres(1)
                elif ci == 6:
                    issue_xres(2)
                elif ci == 10:
                    issue_xres(3)

                gath = gath_tiles.pop(ci)
                if ci in eq_tiles:
                    eq = eq_tiles.pop(ci)
                else:
                    # one-hot(dst) selection matrices for the whole chunk,
                    # stored as fp8 pairs packed in bf16-typed 16-bit cells
                    eq = epool.tile([P, mcg, 64], bf16, tag="eq")
                    nidx = ng + (ng % 2)  # even count; pads are -1
                    cb = col_base[ci]
                    nc.gpsimd.local_scatter(
                        out_ap=eq[:].rearrange("p g d -> p (g d)")[
                            :, : ng * 64
                        ],
                        data_ap=scatidx_sb[
                            :, scat_cols + cb : scat_cols + cb + nidx
                        ].bitcast(bf16),
                        idxs_ap=scatidx_sb[:, cb : cb + nidx],
                        channels=P,
                        num_elems=ng * 64,
                        num_idxs=nidx,
                    )

                for t in range(lo, hi):
                    gt = int(g_per_tile[t])
                    tg = int(slot_base[t]) // P
                    gb = tg - g0  # index into gath buffer / eq (per chunk)
                    psAB = psa_pool.tile([P, 2, 512], f32, space="PSUM")
                    j = 0
                    while j < gt:
                        pair = j + 1 < gt
                        last = (j + 2 if pair else j + 1) >= gt
                        if pair:
                            rr = eq[:, gb + j : gb + j + 2, :].bitcast(f8)
                            for b, fsl in ((0, slice(0, P)), (1, slice(P, F))):
                                nc.tensor.matmul(
                                    out=psAB[:, b, 0:P],
                                    lhsT=gath[:, gb + j : gb + j + 2, fsl],
                                    rhs=rr,
                                    start=(j == 0),
                                    stop=last,
                                    perf_mode=DR,
                                )
                            j += 2
                        else:
                            rr = eq[:, gb + j, :].bitcast(f8)
                            for b, fsl in ((0, slice(0, P)), (1, slice(P, F))):
                                nc.tensor.matmul(
                                    out=psAB[:, b, 0:P],
                                    lhsT=gath[:, gb + j, fsl],
                                    rhs=rr,
                                    start=(j == 0),
                                    stop=last,
                                )
                            j += 1
                    # xa^T tile -> fp8 (scaled): xa8 = fp8(XS * xa)
                    nc.scalar.activation(
                        xaT8_sb[:, :, t * P : (t + 1) * P],
                        psAB[:, :, 0:P],
                        mybir.ActivationFunctionType.Copy,
                        scale=XS,
                    )

                    # dense m-chunk once its tiles are done (tail is ragged)
                    if t % 4 == 3 or t == N_TILES - 1:
                        mc = t // 4
                        ms = mc * 512
                        mw = min(512, NLP - ms)
                        if mc == 0:
                            make_x8(0)
                        psZ = {}
                        for nt in range(2):
                            for z in range(K):
                                ps = psz_pool.tile(
                                    [P, 512], f32, space="PSUM",
                                    tag=f"psz_{z}_{nt}",
                                )
                                psZ[(z, nt)] = ps
                                for part, u in (
                                    (0, xT8_sb),
                                    (1, xaT8_sb),
                                ):
                                    nc.tensor.matmul(
                                        out=ps[:, :mw],
                                        lhsT=wq_sb[:, z * 4 + nt * 2 + part],
                                        rhs=u[:, :, ms : ms + mw],
                                        start=(part == 0),
                                        stop=(part == 1),
                                        perf_mode=DR,
                                    )
                        if mc + 1 <= (NLP - 1) // 512:
                            make_x8(mc + 1)
                        ob = wpool.tile([P, 2, 512], bf16, tag="ob")
                        for nt in range(2):
                            # out_nt = relu(0.5 y0) + relu(0.5 y1) + x
                            tmp = wpool.tile([P, 512], bf16, tag=f"t_{nt}")
                            nc.vector.scalar_tensor_tensor(
                                out=tmp[:, :mw],
                                in0=psZ[(0, nt)][:, :mw],
                                scalar=0.0,
                                in1=xres_sb[:, nt, ms : ms + mw],
                                op0=MAX,
                                op1=ADD,
                            )
                            nc.vector.scalar_tensor_tensor(
                                out=ob[:, nt, :mw],
                                in0=psZ[(1, nt)][:, :mw],
                                scalar=0.0,
                                in1=tmp[:, :mw],
                                op0=MAX,
                                op1=ADD,
                            )
                        nc.scalar.dma_start(
                            out[:, :, ms : ms + mw], ob[:, :, :mw]
                        )

    nc.compile()
    return nc


# --------------------------------------------------------------------------
# Entry point
# --------------------------------------------------------------------------

def kernel(x, edge_index, init_weight, root_weight, bias, _debug=None):
    in_maps, meta = _preprocess(x, edge_index, init_weight, root_weight, bias)
    nc = _build(meta)
    res = run_bass_kernel_spmd(
        nc, in_maps, core_ids=list(range(N_CORES)), **(_debug or {})
    )
    results = res.results if hasattr(res, "results") else res

    out = np.empty((N, F), dtype=np.float32)
    core_of, loc_of = meta["core_of"], meta["loc_of"]
    for r in range(N_CORES):
        mine = np.where(core_of == r)[0]
        o = results[r]["out"].astype(np.float32)  # [P, 2, NLP]
        oc = o.transpose(1, 0, 2).reshape(F, NLP)
        out[mine] = oc[:, loc_of[mine]].T
    return out


if __name__ == "__main__":
    rng = np.random.default_rng(0)
    x = rng.standard_normal((N, F), dtype=np.float32)
    ei = rng.integers(0, N, (2, E))
    iw = rng.standard_normal((K, F, F), dtype=np.float32) * 0.06
    rw = rng.standard_normal((K, F, F), dtype=np.float32) * 0.06
    b = np.zeros((K, 1, F), dtype=np.float32)
    in_maps, meta = _preprocess(x, ei, iw, rw, b)
    print("G =", meta["G"], "EG =", meta["EG"], "chunks =", len(meta["chunks"]))


# revision 18
# speedup vs baseline: 1.0170x; 1.0170x over previous
"""ARMA GNN (single-layer ARMAConv + residual) as a distributed Bass kernel
on 8 TRN2 NeuronCores.

Math (reference):
    deg[d]   = #incoming edges of d;  dinv = deg^-1/2 (0 where deg==0)
    w[e]     = dinv[src_e] * dinv[dst_e]
    xa       = A_hat @ x                (segment-sum of w[e] * x[src_e] into dst_e)
    y_k      = xa @ W_k + x @ V_k + b_k          (assoc: A@(xW) == (A@x)@W)
    out      = x + relu(mean_k relu(y_k)) = x + 0.5*relu(y_0) + 0.5*relu(y_1)
               (outer relu is a no-op on a nonneg sum of relus)

Distribution: nodes are degree-balanced across 8 cores (and across 128-row
tiles within a core). Each core owns its destination nodes and every edge
pointing into them; the halo (x rows its edges read) is shipped per edge
slot, pre-scaled by w_e, as a contiguous wrapped table streamed by plain
DMA. One-hot(dst-position) selection matrices turn the segment-sum into
TensorE matmuls accumulating xa^T tiles in PSUM. The dense matmuls run
entirely in fp8 DoubleRow (2 contraction rows/cycle): z^T = Wq^T [x8;xa8]^T
with x8 = fp8(x/4), xa8 = fp8(xa/4) (scaled PSUM->fp8 copy), Wq = fp8(2W),
so PSUM holds 0.5*y_k exactly and the epilogue is two fused relu+add
(scalar_tensor_tensor max/add) per output half, no extra scaling pass.
One-hot tables for the first chunks are precomputed host-side and DMA'd so
the first matmul does not wait on the gpsimd scatter library warmup.
"""

import sys

for _p in ("/opt/trn_rl_repo", "/opt/pypackages"):
    if _p not in sys.path:
        sys.path.append(_p)

import numpy as np
import ml_dtypes

import concourse.tile as tile
from concourse import bacc, library_config, mybir
from concourse.bass_utils import run_bass_kernel_spmd

BF16 = ml_dtypes.bfloat16
F8 = ml_dtypes.float8_e4m3

# Problem constants (nn_Arma_83330955477199)
N, E, F, K = 50000, 320000, 256, 2
N_CORES = 8
P = 128

# Per-core geometry
NL = N // N_CORES                                # 6250 real nodes per core
N_TILES = (NL + P - 1) // P                      # 49 tiles
NLP = N_TILES * P                                # 6272 padded rows

MAX_CHUNK_GROUPS = 15   # local_scatter cap: num_elems = 15*128 < 2047
NPRE = 3                # leading single-tile chunks with host-built eq
LOOKAHEAD = 5           # gather-DMA prefetch depth (chunks)

XS = 0.25               # fp8 scale on x / xa
WS = 2.0                # fp8 scale on weights  (XS*WS = 0.5 = mean over K)


# --------------------------------------------------------------------------
# Host-side preprocessing: graph partitioning + layout prep
# --------------------------------------------------------------------------

def _preprocess(x, edge_index, init_weight, root_weight, bias):
    src = np.asarray(edge_index[0], dtype=np.int64)
    dst = np.asarray(edge_index[1], dtype=np.int64)
    x = np.asarray(x, dtype=np.float32)

    deg = np.bincount(dst, minlength=N).astype(np.float32)
    dinv = np.where(deg > 0, 1.0 / np.sqrt(np.maximum(deg, 1.0)), 0.0).astype(
        np.float32
    )

    # --- node -> (core, tile, pos): snake-deal by degree for edge balance
    order = np.argsort(-deg, kind="stable")
    core_of = np.empty(N, dtype=np.int32)
    loc_of = np.empty(N, dtype=np.int32)
    n_rounds = N // N_CORES
    fwd = np.arange(N_CORES)
    snake = np.empty((n_rounds, N_CORES), dtype=np.int64)
    snake[0::2] = fwd
    snake[1::2] = fwd[::-1]
    core_of[order] = snake.reshape(-1)
    # within each core: degree-aware bin packing of nodes into tiles so the
    # per-tile edge counts land just under aligned group budgets (the same
    # hi/lo tile split on every core, since the schedule takes the max)
    degi = deg.astype(np.int64)
    percore_e = np.array(
        [degi[core_of == r].sum() for r in range(N_CORES)]
    )
    C_hi, C_lo = 7 * P, 6 * P
    n_hi = int(np.ceil((percore_e.max() + 192 - N_TILES * C_lo) / P))
    n_hi = min(max(n_hi, 0), N_TILES)
    ecaps = np.full(N_TILES, C_lo, dtype=np.int64)
    ecaps[:n_hi] = C_hi
    # ramp-up: tiny leading tiles so the first gathers + eq tables land
    # fast and the PE starts early; their capacity moves to later tiles
    ramp = (2 * P, 4 * P, 6 * P)
    moved = sum(ecaps[i] - ramp[i] for i in range(3))
    ecaps[:3] = ramp
    i = 3
    while moved > 0 and i < N_TILES:
        add = min(P, C_hi + P - ecaps[i], moved)
        if add > 0:
            ecaps[i] += add
            moved -= add
        i += 1
    for r in range(N_CORES):
        nodes_r = order[core_of[order] == r]  # degree-sorted desc
        assert len(nodes_r) == NL
        degs = degi[nodes_r]
        bsum = np.zeros(N_TILES, dtype=np.int64)
        bn = np.zeros(N_TILES, dtype=np.int64)
        for i in range(NL):
            dg = degs[i]
            room = ecaps - bsum
            slots = P - bn
            fit = (slots > 0) & (room >= dg)
            if fit.any():
                sc = np.where(fit, room / np.maximum(slots, 1), -1e18)
                b = int(np.argmax(sc))
            else:
                sc = np.where(slots > 0, (room - dg).astype(float), -1e18)
                b = int(np.argmax(sc))
            loc_of[nodes_r[i]] = b * P + bn[b]
            bsum[b] += dg
            bn[b] += 1

    # --- per-core edge lists (owned by dst core)
    e_core = core_of[dst]
    per_core = []
    for r in range(N_CORES):
        m = e_core == r
        s_r, d_r = src[m], dst[m]
        d_loc = loc_of[d_r]
        per_core.append((s_r, d_r, d_loc // P, d_loc % P))

    # --- static schedule: groups per tile = max over cores
    cnt = np.zeros((N_CORES, N_TILES), dtype=np.int64)
    for r in range(N_CORES):
        cnt[r] = np.bincount(per_core[r][2], minlength=N_TILES)
    g_per_tile = np.maximum(1, (cnt.max(axis=0) + P - 1) // P).astype(np.int64)
    G = int(g_per_tile.sum())
    EG = G * P
    slot_base = np.concatenate([[0], np.cumsum(g_per_tile * P)])

    # --- gather/compute chunks: pack tiles with <= MAX_CHUNK_GROUPS groups
    chunks = [(0, 1), (1, 2), (2, 3)]
    lo = 3
    while lo < N_TILES:
        hi = lo
        gacc = 0
        while hi < N_TILES and (
            gacc + g_per_tile[hi] <= MAX_CHUNK_GROUPS or hi == lo
        ):
            gacc += int(g_per_tile[hi])
            hi += 1
        chunks.append((lo, hi))
        lo = hi
    max_chunk_groups = max(
        int(slot_base[hi] - slot_base[lo]) // P for lo, hi in chunks
    )
    # chunk-local scatter index base per group + even-aligned column bases
    g_chunk_base = np.zeros(G, dtype=np.int64)
    col_base = []
    cols = 0
    for lo, hi in chunks:
        g0 = int(slot_base[lo]) // P
        g1 = int(slot_base[hi]) // P
        g_chunk_base[g0:g1] = g0
        col_base.append(cols)
        ng = g1 - g0
        cols += ng + (ng % 2)  # always even per chunk -> 4B-aligned bases
    scat_cols = cols

    npre_g = int(slot_base[NPRE]) // P  # groups covered by host-built eq

    # --- per-core device inputs
    in_maps = []
    for r in range(N_CORES):
        s_r, d_r, t_r, p_r = per_core[r]

        slot_src = np.zeros(EG, dtype=np.int64)
        slot_pos = np.full(EG, -1, dtype=np.int64)
        slot_w = np.zeros(EG, dtype=np.float32)
        eorder = np.argsort(t_r, kind="stable")
        ts_sorted = t_r[eorder]
        starts = np.searchsorted(ts_sorted, np.arange(N_TILES))
        ends = np.searchsorted(ts_sorted, np.arange(N_TILES) + 1)
        for t in range(N_TILES):
            es = eorder[starts[t]:ends[t]]
            b = slot_base[t]
            slot_src[b:b + len(es)] = s_r[es]
            slot_pos[b:b + len(es)] = p_r[es]
            slot_w[b:b + len(es)] = dinv[d_r[es]] * dinv[s_r[es]]

        # local_scatter: one-hot stored as fp8 byte pairs in 16-bit cells;
        # cell = (group - chunk_base)*64 + dstpos//2, byte = dstpos parity.
        # cols [0, scat_cols) are indices, [scat_cols, 2*scat_cols) data.
        gidx = np.arange(G)
        pos = slot_pos.reshape(G, P)
        sidx = np.where(
            pos >= 0,
            (gidx - g_chunk_base)[:, None] * 64 + pos // 2,
            -1,
        ).astype(np.int16)  # [G, 128]
        sdat = np.where(pos % 2 == 0, 0x0038, 0x3800).astype(np.int16)
        scatidx = np.full((P, 2 * scat_cols), -1, dtype=np.int16)
        scatidx[:, scat_cols:] = 0  # data half: pads must stay finite as bf16
        for ci, (clo, chi) in enumerate(chunks):
            cg0 = int(slot_base[clo]) // P
            cg1 = int(slot_base[chi]) // P
            cb = col_base[ci]
            scatidx[:, cb : cb + cg1 - cg0] = sidx[cg0:cg1].T
            scatidx[:, scat_cols + cb : scat_cols + cb + cg1 - cg0] = (
                sdat[cg0:cg1].T
            )

        # host-materialized eq for the first NPRE chunks (what local_scatter
        # would produce: zeros + one fp8(1.0) byte per real slot)
        eq01 = np.zeros((P, npre_g, 64), dtype=np.int16)
        for g in range(npre_g):
            pg = pos[g]  # [128] dst positions (or -1)
            for p in range(P):
                if pg[p] >= 0:
                    eq01[p, g, pg[p] // 2] = (
                        0x0038 if pg[p] % 2 == 0 else 0x3800
                    )

        # per-slot message rows, wrapped so each partition's stream is
        # contiguous in DRAM: slots[p, c, :] = row (c*128 + p)
        slots = np.ascontiguousarray(
            (x[slot_src] * slot_w[:, None])
            .astype(F8)
            .reshape(G, P, F)
            .transpose(1, 0, 2)
        )  # [128, G, F]

        mine = np.where(core_of == r)[0]
        x_core = np.zeros((NLP, F), dtype=np.float32)
        x_core[loc_of[mine]] = x[mine]
        xcT = np.ascontiguousarray(
            x_core.T.reshape(2, P, NLP).transpose(1, 0, 2)
        )  # [p, block, m] fp32; f = block*128 + p
        xres = xcT.astype(BF16)

        in_maps.append(
            {
                "slots": slots,
                "scatidx": scatidx,
                "xres": xres,
                "eq01": eq01.view(BF16),
            }
        )

    # replicated fp8 weights: wq[p, z*4 + nt*2 + part, jj, n]
    #   = WS * Wpart_z[jj*128 + p, nt*128 + n]   (part 0 = root/x, 1 = init/xa)
    wq = np.zeros((P, 8, 2, P), dtype=np.float32)
    for z in range(K):
        for part, wmat in ((0, root_weight[z]), (1, init_weight[z])):
            wm = np.asarray(wmat, dtype=np.float32)
            for nt in range(2):
                for jj in range(2):
                    wq[:, z * 4 + nt * 2 + part, jj, :] = (
                        WS * wm[jj * P : (jj + 1) * P, nt * P : (nt + 1) * P]
                    )
    wq = np.ascontiguousarray(wq).astype(F8)

    bias_np = np.asarray(bias, dtype=np.float32)
    has_bias = bool(np.any(bias_np != 0.0))
    assert not has_bias, "nonzero bias not implemented (reference uses zeros)"

    for m in in_maps:
        m["wq"] = wq

    meta = {
        "g_per_tile": g_per_tile,
        "slot_base": slot_base,
        "G": G,
        "EG": EG,
        "chunks": chunks,
        "col_base": col_base,
        "scat_cols": scat_cols,
        "max_chunk_groups": max_chunk_groups,
        "npre_g": npre_g,
        "core_of": core_of,
        "loc_of": loc_of,
    }
    return in_maps, meta


# --------------------------------------------------------------------------
# Device kernel builder
# --------------------------------------------------------------------------

def _build(meta):
    g_per_tile = meta["g_per_tile"]
    slot_base = meta["slot_base"]
    G, EG = meta["G"], meta["EG"]
    chunks = meta["chunks"]
    col_base = meta["col_base"]
    scat_cols = meta["scat_cols"]
    mcg = meta["max_chunk_groups"]
    npre_g = meta["npre_g"]

    nc = bacc.Bacc(
        "TRN2", target_bir_lowering=False, debug=False, num_devices=N_CORES
    )
    bf16 = mybir.dt.bfloat16
    f32 = mybir.dt.float32
    f8 = mybir.dt.float8e4
    i16 = mybir.dt.int16
    DR = mybir.MatmulPerfMode.DoubleRow
    MAX = mybir.AluOpType.max
    ADD = mybir.AluOpType.add

    slots = nc.declare_dram_parameter("slots", [P, G, F], f8, isOutput=False)
    scatidx = nc.declare_dram_parameter(
        "scatidx", [P, 2 * scat_cols], i16, isOutput=False
    )
    xres = nc.declare_dram_parameter("xres", [P, 2, NLP], bf16, isOutput=False)
    wq = nc.declare_dram_parameter("wq", [P, 8, 2, P], f8, isOutput=False)
    eq01 = nc.declare_dram_parameter(
        "eq01", [P, npre_g, 64], bf16, isOutput=False
    )
    out = nc.declare_dram_parameter("out", [P, 2, NLP], bf16, isOutput=True)

    n_chunks = len(chunks)

    def chunk_g(ci):
        lo, hi = chunks[ci]
        s0, s1 = int(slot_base[lo]), int(slot_base[hi])
        return s0 // P, (s1 - s0) // P  # (first group, #groups)

    with tile.TileContext(nc) as tc:
        with (
            tc.tile_pool(name="const", bufs=1) as cpool,
            tc.tile_pool(name="gath", bufs=LOOKAHEAD + 1) as gpool,
            tc.tile_pool(name="eq", bufs=4) as epool,
            tc.tile_pool(name="work", bufs=3) as wpool,
            tc.tile_pool(name="psA", bufs=2, space="PSUM") as psa_pool,
            tc.tile_pool(name="psZ", bufs=1, space="PSUM") as psz_pool,
        ):
            nc.gpsimd.load_library(library_config.local_scatter)

            scatidx_sb = cpool.tile([P, 2 * scat_cols], i16)
            wq_sb = cpool.tile([P, 8, 2, P], f8)
            xres_sb = cpool.tile([P, 2, NLP], bf16)
            xT8_sb = cpool.tile([P, 2, NLP], f8)
            xaT8_sb = cpool.tile([P, 2, NLP], f8)

            gath_tiles = {}

            def issue_gath(ci):
                if ci >= n_chunks:
                    return
                g0, ng = chunk_g(ci)
                gt = gpool.tile([P, mcg, F], f8, tag="gath")
                nc.sync.dma_start(gt[:, :ng, :], slots[:, g0 : g0 + ng, :])
                gath_tiles[ci] = gt

            # xres streamed in m-chunk-aligned pieces; x8 = fp8(XS * x)
            # derived on chip one m-chunk ahead of its dense matmuls
            XQ = [(0, 1536), (1536, 3072), (3072, 4608), (4608, NLP)]
            xq_of_mc = lambda mc: min(mc // 3, 3)

            def issue_xres(q):
                qs, qe = XQ[q]
                nc.scalar.dma_start(
                    xres_sb[:, :, qs:qe], xres[:, :, qs:qe]
                )

            def make_x8(mc):
                ms = mc * 512
                mw = min(512, NLP - ms)
                nc.scalar.activation(
                    xT8_sb[:, :, ms : ms + mw],
                    xres_sb[:, :, ms : ms + mw],
                    mybir.ActivationFunctionType.Copy,
                    scale=XS,
                )

            # priority startup: gathers on sync (alone on that queue),
            # everything else on scalar
            for ci in range(LOOKAHEAD):
                issue_gath(ci)
            eq_tiles = {}
            for ci in range(min(NPRE, n_chunks)):
                g0, ng = chunk_g(ci)
                et = epool.tile([P, mcg, 64], bf16, tag="eq")
                nc.scalar.dma_start(
                    et[:, :ng, :], eq01[:, g0 : g0 + ng, :]
                )
                eq_tiles[ci] = et
            nc.scalar.dma_start(scatidx_sb[:], scatidx[:, :])
            nc.scalar.dma_start(wq_sb[:], wq[:, :, :, :])
            issue_xres(0)


            for ci in range(n_chunks):
                lo, hi = chunks[ci]
                g0, ng = chunk_g(ci)

                issue_gath(ci + LOOKAHEAD)
                if ci == 2:
                    issue_xres(1)
                elif ci == 6:
                    issue_xres(2)
                elif ci == 10:
                    issue_xres(3)

                gath = gath_tiles.pop(ci)
                if ci in eq_tiles:
                    eq = eq_tiles.pop(ci)
                else:
                    # one-hot(dst) selection matrices for the whole chunk,
                    # stored as fp8 pairs packed in bf16-typed 16-bit cells
                    eq = epool.tile([P, mcg, 64], bf16, tag="eq")
                    nidx = ng + (ng % 2)  # even count; pads are -1
                    cb = col_base[ci]
                    nc.gpsimd.local_scatter(
                        out_ap=eq[:].rearrange("p g d -> p (g d)")[
                            :, : ng * 64
                        ],
                        data_ap=scatidx_sb[
                            :, scat_cols + cb : scat_cols + cb + nidx
                        ].bitcast(bf16),
                        idxs_ap=scatidx_sb[:, cb : cb + nidx],
                        channels=P,
                        num_elems=ng * 64,
                        num_idxs=nidx,
                    )

                for t in range(lo, hi):
                    gt = int(g_per_tile[t])
                    tg = int(slot_base[t]) // P
                    gb = tg - g0  # index into gath buffer / eq (per chunk)
                    psAB = psa_pool.tile([P, 2, 512], f32, space="PSUM")
                    j = 0
                    while j < gt:
                        pair = j + 1 < gt
                        last = (j + 2 if pair else j + 1) >= gt
                        if pair:
                            rr = eq[:, gb + j : gb + j + 2, :].bitcast(f8)
                            for b, fsl in ((0, slice(0, P)), (1, slice(P, F))):
                                nc.tensor.matmul(
                                    out=psAB[:, b, 0:P],
                                    lhsT=gath[:, gb + j : gb + j + 2, fsl],
                                    rhs=rr,
                                    start=(j == 0),
                                    stop=last,
                                    perf_mode=DR,
                                )
                            j += 2
                        else:
                            rr = eq[:, gb + j, :].bitcast(f8)
                            for b, fsl in ((0, slice(0, P)), (1, slice(P, F))):
                                nc.tensor.matmul(
                                    out=psAB[:, b, 0:P],
                                    lhsT=gath[:, gb + j, fsl],
                                    rhs=rr,
                                    start=(j == 0),
                                    stop=last,
                                )
                            j += 1
                    # xa^T tile -> fp8 (scaled): xa8 = fp8(XS * xa)
                    nc.scalar.activation(
                        xaT8_sb[:, :, t * P : (t + 1) * P],
                        psAB[:, :, 0:P],
                        mybir.ActivationFunctionType.Copy,
                        scale=XS,
                    )

                    # dense m-chunk once its tiles are done (tail is ragged)
                    if t % 4 == 3 or t == N_TILES - 1:
                        mc = t // 4
                        ms = mc * 512
                        mw = min(512, NLP - ms)
                        if mc == 0:
                            make_x8(0)
                        psZ = {}
                        for nt in range(2):
                            for z in range(K):
                                ps = psz_pool.tile(
                                    [P, 512], f32, space="PSUM",
                                    tag=f"psz_{z}_{nt}",
                                )
                                psZ[(z, nt)] = ps
                                for part, u in (
                                    (0, xT8_sb),
                                    (1, xaT8_sb),
                                ):
                                    nc.tensor.matmul(
                                        out=ps[:, :mw],
                                        lhsT=wq_sb[:, z * 4 + nt * 2 + part],
                                        rhs=u[:, :, ms : ms + mw],
                                        start=(part == 0),
                                        stop=(part == 1),
                                        perf_mode=DR,
                                    )
                        if mc + 1 <= (NLP - 1) // 512:
                            make_x8(mc + 1)
                        ob = wpool.tile([P, 2, 512], bf16, tag="ob")
                        for nt in range(2):
                            # out_nt = relu(0.5 y0) + relu(0.5 y1) + x
                            tmp = wpool.tile([P, 512], bf16, tag=f"t_{nt}")
                            nc.vector.scalar_tensor_tensor(
                                out=tmp[:, :mw],
                                in0=psZ[(0, nt)][:, :mw],
                                scalar=0.0,
                                in1=xres_sb[:, nt, ms : ms + mw],
                                op0=MAX,
                                op1=ADD,
                            )
                            nc.vector.scalar_tensor_tensor(
                                out=ob[:, nt, :mw],
                                in0=psZ[(1, nt)][:, :mw],
                                scalar=0.0,
                                in1=tmp[:, :mw],
                                op0=MAX,
                                op1=ADD,
                            )
                        nc.scalar.dma_start(
                            out[:, :, ms : ms + mw], ob[:, :, :mw]
                        )

    nc.compile()
    return nc


# --------------------------------------------------------------------------
# Entry point
# --------------------------------------------------------------------------

def kernel(x, edge_index, init_weight, root_weight, bias, _debug=None):
    in_maps, meta = _preprocess(x, edge_index, init_weight, root_weight, bias)
    nc = _build(meta)
    res = run_bass_kernel_spmd(
        nc, in_maps, core_ids=list(range(N_CORES)), **(_debug or {})
    )
    results = res.results if hasattr(res, "results") else res

    out = np.empty((N, F), dtype=np.float32)
    core_of, loc_of = meta["core_of"], meta["loc_of"]
    for r in range(N_CORES):
        mine = np.where(core_of == r)[0]
        o = results[r]["out"].astype(np.float32)  # [P, 2, NLP]
        oc = o.transpose(1, 0, 2).reshape(F, NLP)
        out[mine] = oc[:, loc_of[mine]].T
    return out


if __name__ == "__main__":
    rng = np.random.default_rng(0)
    x = rng.standard_normal((N, F), dtype=np.float32)
    ei = rng.integers(0, N, (2, E))
    iw = rng.standard_normal((K, F, F), dtype=np.float32) * 0.06
    rw = rng.standard_normal((K, F, F), dtype=np.float32) * 0.06
    b = np.zeros((K, 1, F), dtype=np.float32)
    in_maps, meta = _preprocess(x, ei, iw, rw, b)
    print("G =", meta["G"], "EG =", meta["EG"], "chunks =", len(meta["chunks"]))
